# revision 1
# baseline (speedup 1.0000x reference)
"""Trainium2 Bass kernel for HierarchicalCSNet (8 groups, 256x256, G-fused chain).

Strategy: spatial row-sharding across 8 NeuronCores. Core c owns image rows
[32c, 32c+32) and recomputes shrinking halo margins locally (zero collectives).
The tiny head (strided sample conv + 1x1 upsample + block-scatter reshape) is
computed on host; everything from the first 3x3 conv onward runs on device as
fp32r tap-accumulated matmuls.

Slot grid per core: slot s in [0,56) <-> global row 32c - 12 + s. All on-chip
feature rows are stored at pitch 258 (1 zero pad col each side). Margins:
feature_m needs margin M_m = 11 - m, h_m needs H_m = M_m + 1 (H_0 = 11).
Image-edge cores zero their out-of-image margin rows via per-core mask scalars.
"""
import sys, os
import numpy as np

for _p in ("/opt/trn_rl_repo", os.path.expanduser("~/.axon_site/_ro/trn_rl_repo")):
    if os.path.isdir(_p) and _p not in sys.path:
        sys.path.append(_p)

G, BS = 8, 32
H = W = 256
PITCH = 258
NSLOT = 54          # slots [1,55) stored in F/TA (row = slot-1)
TB_BASE = 9
TB_ROWS = 38        # slots [9,47) stored in TB (row = slot-9)


def _h_range(m):
    return (1, 55) if m == 0 else (m, 56 - m)


def _fus_range(m):
    return (m + 1, 55 - m)


def _chunks():
    """(m, s0) list for h-conv tiles, in program order."""
    out = []
    for m in range(G):
        lo, hi = _h_range(m)
        for s0 in range(lo, hi, 2):
            out.append((m, s0))
    return out


_CHUNKS = _chunks()
NCHUNK = len(_CHUNKS)

_BUILT = None


def _build_program(reps=1):
    import concourse.bacc as bacc
    import concourse.mybir as mybir
    import concourse.tile as tile

    f32 = mybir.dt.float32
    f32r = mybir.dt.float32r
    PRELU = mybir.ActivationFunctionType.Prelu
    COPY = mybir.ActivationFunctionType.Copy

    nc = bacc.Bacc("TRN2", target_bir_lowering=False)
    r9_d = nc.dram_tensor("r9", [NCHUNK * 9, 516], f32r, kind="ExternalInput")
    wh_d = nc.dram_tensor("wh", [9, G * 64], f32r, kind="ExternalInput")
    wf_d = nc.dram_tensor("wf", [7 * 128, 576], f32r, kind="ExternalInput")
    wt_d = nc.dram_tensor("wt", [G * 128, 1152], f32r, kind="ExternalInput")
    w5_d = nc.dram_tensor("w5", [128, G * 6], f32r, kind="ExternalInput")
    bb_d = nc.dram_tensor("bb", [64, 39], f32, kind="ExternalInput")
    aa_d = nc.dram_tensor("aa", [64, 39], f32, kind="ExternalInput")
    mm_d = nc.dram_tensor("mm", [128, 2], f32, kind="ExternalInput")
    o_d = nc.dram_tensor("o", [G, 32 * 256], f32, kind="ExternalOutput")

    with tile.TileContext(nc) as tc:
        with tc.tile_pool(name="const", bufs=1) as cst, \
             tc.tile_pool(name="big", bufs=1) as big, \
             tc.tile_pool(name="wfp", bufs=2) as wfp, \
             tc.tile_pool(name="wtp", bufs=2) as wtp, \
             tc.tile_pool(name="r9p", bufs=4) as r9p, \
             tc.tile_pool(name="o5p", bufs=2) as o5p, \
             tc.tile_pool(name="ps", bufs=6, space="PSUM") as ps, \
             tc.tile_pool(name="p5", bufs=2, space="PSUM") as p5p:

            wh_t = cst.tile([9, G * 64], f32r)
            w5_t = cst.tile([128, G * 6], f32r)
            bb_t = cst.tile([64, 39], f32)
            aa_t = cst.tile([64, 39], f32)
            mm_t = cst.tile([128, 2], f32)
            F = big.tile([128, NSLOT * PITCH], f32r)
            TA = big.tile([128, NSLOT * PITCH], f32r)
            TB = big.tile([128, TB_ROWS * PITCH], f32r)

            nc.sync.dma_start(wh_t[:], wh_d[:])
            nc.sync.dma_start(w5_t[:], w5_d[:])
            nc.sync.dma_start(bb_t[:], bb_d[:])
            nc.sync.dma_start(aa_t[:], aa_d[:])
            nc.sync.dma_start(mm_t[:], mm_d[:])
            nc.vector.memset(F[:].bitcast(f32), 0.0)
            nc.vector.memset(TA[:].bitcast(f32), 0.0)
            nc.vector.memset(TB[:].bitcast(f32), 0.0)

            Fv = F[:].rearrange("p (r x) -> p r x", x=PITCH)
            TAv = TA[:].rearrange("p (r x) -> p r x", x=PITCH)
            TBv = TB[:].rearrange("p (r x) -> p r x", x=PITCH)

            def mask(view, base, mlo, mhi, stacked=False, nrows=NSLOT):
                # zero out-of-image rows: top slots [mlo,12) with mm[:,0],
                # bottom slots [44,mhi) with mm[:,1] (no-op on interior cores)
                for (lo, hi, col) in ((mlo, 12, 0), (44, mhi, 1)):
                    if hi <= lo:
                        continue
                    nc.vector.tensor_scalar_mul(
                        view[0:64, lo - base:hi - base, :],
                        view[0:64, lo - base:hi - base, :],
                        mm_t[0:64, col:col + 1])
                if not stacked:
                    return
                # upper half holds rows shifted by +1 slot
                for (lo, hi, col) in ((mlo, 12, 0), (44, mhi, 1)):
                    rlo = max(0, lo - base - 1)
                    rhi = min(nrows, hi - base - 1)
                    if rhi <= rlo:
                        continue
                    nc.vector.tensor_scalar_mul(
                        view[64:128, rlo:rhi, :],
                        view[64:128, rlo:rhi, :],
                        mm_t[64:128, col:col + 1])

            def stack_dma(buf, base, s0, nrows):
                # buf[64:128, r] := buf[0:64, r+1] for the rows enabled by the
                # freshly written tile (slots s0, s0+1)
                d0 = max(0, s0 - base - 1)
                d1 = min(nrows - 1, s0 - base + 1)
                if d1 <= d0:
                    return
                nc.sync.dma_start(
                    buf[64:128, d0 * PITCH:d1 * PITCH],
                    buf[0:64, (d0 + 1) * PITCH:(d1 + 1) * PITCH])

            for _rep in range(reps):
              chunk_idx = 0
              for m in range(G):
                # --- stream this group's tail weights ---
                wt_t = wtp.tile([128, 1152], f32r, tag="wt")
                nc.sync.dma_start(wt_t[:], wt_d[m * 128:(m + 1) * 128, :])

                # --- h_m: K=9 matmuls from streamed r9 chunks ---
                h_lo, h_hi = _h_range(m)
                hdst, hbase = (TAv, 1) if m == 0 else (Fv, 1)
                for s0 in range(h_lo, h_hi, 2):
                    r9c = r9p.tile([9, 2 * 258], f32r, tag="r9")
                    nc.sync.dma_start(
                        r9c[:], r9_d[chunk_idx * 9:(chunk_idx + 1) * 9, :])
                    chunk_idx += 1
                    r9v = r9c[:].rearrange("p (r x) -> p r x", x=258)
                    pt = ps.tile([64, 512], f32, tag="ps")
                    nc.tensor.matmul(pt[:], wh_t[:, m * 64:(m + 1) * 64],
                                     r9v[0:9, 0:2, 1:257], start=True, stop=True)
                    nc.scalar.activation(
                        hdst[0:64, s0 - hbase:s0 - hbase + 2, 1:257], pt[:],
                        PRELU, bias=bb_t[:, m:m + 1], scale=1.0,
                        alpha=aa_t[:, m:m + 1])
                    if m == 0:
                        stack_dma(TA, 1, s0, NSLOT)
                mask(hdst, hbase, h_lo, h_hi, stacked=(m == 0))

                # --- fusion m (m>=1): K=128 from F = [h_m | feature_{m-1}] ---
                if m >= 1:
                    wf_t = wfp.tile([128, 576], f32r, tag="wf")
                    nc.sync.dma_start(
                        wf_t[:], wf_d[(m - 1) * 128:m * 128, :])
                    f_lo, f_hi = _fus_range(m)
                    for s0 in range(f_lo, f_hi, 2):
                        pt = ps.tile([64, 512], f32, tag="ps")
                        for t in range(9):
                            dy, dx = t // 3, t % 3
                            rr = s0 + dy - 1 - 1
                            nc.tensor.matmul(
                                pt[:], wf_t[:, t * 64:(t + 1) * 64],
                                Fv[0:128, rr:rr + 2, dx:dx + 256],
                                start=(t == 0), stop=(t == 8))
                        nc.scalar.activation(
                            TAv[0:64, s0 - 1:s0 + 1, 1:257], pt[:],
                            PRELU, bias=bb_t[:, 8 + m - 1:8 + m],
                            scale=1.0, alpha=aa_t[:, 8 + m - 1:8 + m])
                        stack_dma(TA, 1, s0, NSLOT)
                    mask(TAv, 1, f_lo, f_hi, stacked=True)

                # --- feature_m (in TA) -> F[64:128] for next fusion ---
                if m < G - 1:
                    lo, hi = (1, 55) if m == 0 else _fus_range(m)
                    nc.sync.dma_start(
                        F[64:128, (lo - 1) * PITCH:(hi - 1) * PITCH],
                        TA[0:64, (lo - 1) * PITCH:(hi - 1) * PITCH])

                # --- tails (dy-packed: 3x K=128 + 3x K=64 per tile) ---
                def tconv(src_v, src_base, dst_v, dst_base, dst_buf, dst_rows,
                          lo, hi, cv, bcol):
                    for s0 in range(lo, hi, 2):
                        pt = ps.tile([64, 512], f32, tag="ps")
                        for j in range(6):
                            dx = j % 3
                            c0 = (cv * 6 + j) * 64
                            if j < 3:   # dy=0 (lower) + dy=1 (stacked upper)
                                rr = s0 - 1 - src_base
                                nc.tensor.matmul(
                                    pt[:], wt_t[:, c0:c0 + 64],
                                    src_v[0:128, rr:rr + 2, dx:dx + 256],
                                    start=(j == 0), stop=False)
                            else:       # dy=2 from lower half
                                rr = s0 + 1 - src_base
                                nc.tensor.matmul(
                                    pt[:], wt_t[0:64, c0:c0 + 64],
                                    src_v[0:64, rr:rr + 2, dx:dx + 256],
                                    start=False, stop=(j == 5))
                        nc.scalar.activation(
                            dst_v[0:64, s0 - dst_base:s0 - dst_base + 2, 1:257],
                            pt[:], PRELU, bias=bb_t[:, bcol:bcol + 1],
                            scale=1.0, alpha=aa_t[:, bcol:bcol + 1])
                        stack_dma(dst_buf, dst_base, s0, dst_rows)

                tconv(TAv, 1, TBv, TB_BASE, TB, TB_ROWS, 9, 47, 0, 15 + m)
                mask(TBv, TB_BASE, 9, 47, stacked=True, nrows=TB_ROWS)
                tconv(TBv, TB_BASE, TAv, 1, TA, NSLOT, 10, 46, 1, 23 + m)
                mask(TAv, 1, 10, 46, stacked=True)
                tconv(TAv, 1, TBv, TB_BASE, TB, TB_ROWS, 11, 45, 2, 31 + m)
                mask(TBv, TB_BASE, 11, 45, stacked=True, nrows=TB_ROWS)

                # --- t5: M=1, dy-packed like the tails ---
                for s0 in range(12, 44, 2):
                    pt5 = p5p.tile([1, 512], f32, tag="p5")
                    for j in range(6):
                        dx = j % 3
                        c5 = m * 6 + j
                        if j < 3:
                            rr = s0 - 1 - TB_BASE
                            nc.tensor.matmul(
                                pt5[:], w5_t[:, c5:c5 + 1],
                                TBv[0:128, rr:rr + 2, dx:dx + 256],
                                start=(j == 0), stop=False)
                        else:
                            rr = s0 + 1 - TB_BASE
                            nc.tensor.matmul(
                                pt5[:], w5_t[0:64, c5:c5 + 1],
                                TBv[0:64, rr:rr + 2, dx:dx + 256],
                                start=False, stop=(j == 5))
                    o5 = o5p.tile([1, 512], f32, tag="o5")
                    nc.scalar.activation(o5[:], pt5[:], COPY)
                    nc.sync.dma_start(
                        o_d[m, (s0 - 12) * 256:(s0 - 10) * 256], o5[:])

    nc.compile()
    return nc


def _get_program():
    global _BUILT
    if _BUILT is None:
        _BUILT = _build_program()
    return _BUILT


def _host_heads(x, sample_w, up_w, up_b):
    """r[m] (256x256) for all groups, float32."""
    X = x[0, 0].reshape(8, 32, 8, 32).astype(np.float64)
    R = np.empty((G, H, W), np.float32)
    for m in range(G):
        S = np.einsum('ipjq,cpq->cij', X, sample_w[m, :, 0].astype(np.float64))
        U = np.einsum('cij,uc->uij', S, up_w[m, :, :, 0, 0].astype(np.float64))
        U = U + up_b[m].astype(np.float64)[:, None, None]
        R[m] = U.reshape(32, 32, 8, 8).transpose(2, 0, 3, 1).reshape(256, 256)
    return R


def _build_r9(R):
    """Per-core prestacked h-conv rhs: [8][NCHUNK*9, 516] float32."""
    from numpy.lib.stride_tricks import sliding_window_view
    rp = np.zeros((G, H + 26, W + 4), np.float32)   # rows g+13, cols x+2
    rp[:, 13:13 + H, 2:2 + W] = R
    out = np.empty((8, NCHUNK, 9, 516), np.float32)
    k0 = 0
    for m in range(G):
        lo, hi = _h_range(m)
        s0s = np.arange(lo, hi, 2)
        SW = sliding_window_view(rp[m], (2, 258))
        for t in range(9):
            dy, dx = t // 3, t % 3
            g0 = (32 * np.arange(8))[:, None] + s0s[None, :] + dy
            out[:, k0:k0 + len(s0s), t] = SW[g0, dx].reshape(8, len(s0s), 516)
        k0 += len(s0s)
    return out.reshape(8, NCHUNK * 9, 516)


_EXEC = None


def _get_executor():
    """Persistent jitted shard_map executor over 8 cores (mirrors
    bass2jax.run_bass_via_pjrt, but reusable for repeat timing)."""
    global _EXEC
    if _EXEC is not None:
        return _EXEC
    import jax
    import jax.numpy as jnp
    from jax.sharding import Mesh, PartitionSpec
    from jax.experimental.shard_map import shard_map
    import concourse.mybir as mybir
    from concourse import bass2jax

    nc = _get_program()
    bass2jax.install_neuronx_cc_hook()

    part_name = nc.partition_id_tensor.name if nc.partition_id_tensor else None
    in_names, out_names, out_avals, zero_shapes = [], [], [], []
    for alloc in nc.m.functions[0].allocations:
        if not isinstance(alloc, mybir.MemoryLocationSet):
            continue
        name = alloc.memorylocations[0].name
        if alloc.kind == "ExternalInput":
            if name != part_name:
                in_names.append(name)
        elif alloc.kind == "ExternalOutput":
            out_names.append(name)
            shape = tuple(alloc.tensor_shape)
            dtype = mybir.dt.np(alloc.dtype)
            out_avals.append(jax.core.ShapedArray(shape, dtype))
            zero_shapes.append((shape, dtype))
    n_params = len(in_names)
    all_names = in_names + out_names
    if part_name is not None:
        all_names = all_names + [part_name]

    def _body(*args):
        operands = list(args)
        if part_name is not None:
            operands.append(bass2jax.partition_id_tensor())
        outs = bass2jax._bass_exec_p.bind(
            *operands,
            out_avals=tuple(out_avals),
            in_names=tuple(all_names),
            out_names=tuple(out_names),
            lowering_input_output_aliases=(),
            sim_require_finite=True,
            sim_require_nnan=True,
            nc=nc,
        )
        return tuple(outs)

    devices = jax.devices()[:8]
    mesh = Mesh(np.asarray(devices), ("core",))
    n_outs = len(out_names)
    sharded = jax.jit(
        shard_map(_body, mesh=mesh,
                  in_specs=(PartitionSpec("core"),) * (n_params + n_outs),
                  out_specs=(PartitionSpec("core"),) * n_outs,
                  check_rep=False),
        keep_unused=True)
    _EXEC = (sharded, in_names, out_names, zero_shapes)
    return _EXEC


def _prep_device_args(in_maps):
    import jax
    sharded, in_names, out_names, zero_shapes = _get_executor()
    concat_in = [np.concatenate([in_maps[c][n] for c in range(8)], axis=0)
                 for n in in_names]
    concat_zero = [np.zeros((8 * s[0],) + tuple(s[1:]), d)
                   for (s, d) in zero_shapes]
    return [jax.device_put(a) for a in concat_in + concat_zero]


def _run(in_maps):
    sharded, in_names, out_names, zero_shapes = _get_executor()
    args = _prep_device_args(in_maps)
    outs = sharded(*args)
    res = []
    for c in range(8):
        res.append({n: np.asarray(outs[i]).reshape((8,) + zero_shapes[i][0])[c]
                    for i, n in enumerate(out_names)})
    return res


def bench(in_maps, iters=5):
    """Device-resident repeat timing of the sharded program. Returns
    (best_seconds, times)."""
    import time as _t
    sharded, *_ = _get_executor()
    args = _prep_device_args(in_maps)
    r = sharded(*args)
    [x.block_until_ready() for x in r]
    times = []
    for _ in range(iters):
        t0 = _t.perf_counter()
        r = sharded(*args)
        [x.block_until_ready() for x in r]
        times.append(_t.perf_counter() - t0)
    return min(times), times


def _make_executor(nc):
    import jax
    from jax.sharding import Mesh, PartitionSpec
    from jax.experimental.shard_map import shard_map
    from concourse import bass2jax
    import concourse.mybir as mybir

    bass2jax.install_neuronx_cc_hook()
    part_name = nc.partition_id_tensor.name if nc.partition_id_tensor else None
    in_names, out_names, out_avals, zero_shapes = [], [], [], []
    for alloc in nc.m.functions[0].allocations:
        if not isinstance(alloc, mybir.MemoryLocationSet):
            continue
        name = alloc.memorylocations[0].name
        if alloc.kind == "ExternalInput":
            if name != part_name:
                in_names.append(name)
        elif alloc.kind == "ExternalOutput":
            out_names.append(name)
            shape = tuple(alloc.tensor_shape)
            dtype = mybir.dt.np(alloc.dtype)
            out_avals.append(jax.core.ShapedArray(shape, dtype))
            zero_shapes.append((shape, dtype))
    all_names = in_names + out_names + ([part_name] if part_name else [])

    def _body(*args):
        operands = list(args)
        if part_name:
            operands.append(bass2jax.partition_id_tensor())
        return tuple(bass2jax._bass_exec_p.bind(
            *operands, out_avals=tuple(out_avals), in_names=tuple(all_names),
            out_names=tuple(out_names), lowering_input_output_aliases=(),
            sim_require_finite=True, sim_require_nnan=True, nc=nc))

    mesh = Mesh(np.asarray(jax.devices()[:8]), ("core",))
    n = len(in_names) + len(out_names)
    sharded = jax.jit(shard_map(_body, mesh=mesh,
                                in_specs=(PartitionSpec("core"),) * n,
                                out_specs=(PartitionSpec("core"),) * len(out_names),
                                check_rep=False), keep_unused=True)
    return sharded, in_names, out_names, zero_shapes


def bench_reps(in_maps, iters=5):
    """Time a 2x-unrolled variant of the program against the 1x program;
    the wall-clock difference is one full device execution, free of the
    fixed axon-RPC dispatch overhead (~100ms) that dominates single calls."""
    import time as _t
    import jax
    results = {}
    for reps in (1, 2):
        nc = _get_program() if reps == 1 else _build_program(reps=2)
        sharded, in_names, out_names, zero_shapes = _make_executor(nc)
        concat_in = [np.concatenate([in_maps[c][n] for c in range(8)], axis=0)
                     for n in in_names]
        concat_zero = [np.zeros((8 * s[0],) + tuple(s[1:]), d)
                       for (s, d) in zero_shapes]
        args = [jax.device_put(a) for a in concat_in + concat_zero]
        r = sharded(*args); [x.block_until_ready() for x in r]
        ts = []
        for _ in range(iters):
            t0 = _t.perf_counter()
            r = sharded(*args)
            [x.block_until_ready() for x in r]
            ts.append(_t.perf_counter() - t0)
        ts.sort()
        results[reps] = ts
    # median-based difference is more robust to tunnel jitter than min
    import statistics
    d = statistics.median(results[2]) - statistics.median(results[1])
    return max(d, 0.0), results


def build_in_maps(x, sample_w, up_w, up_b, h1_w, h1_b, h1_a, fus_w, fus_b,
                  fus_a, t2_w, t2_b, t2_a, t3_w, t3_b, t3_a, t4_w, t4_b,
                  t4_a, t5_w, t5_b):

    R = _host_heads(x, sample_w, up_w, up_b)
    r9 = _build_r9(R)

    wh = np.ascontiguousarray(
        h1_w[:, :, 0].reshape(G, 64, 9).transpose(2, 0, 1).reshape(9, G * 64))
    # fusion lhsT rows 0:64 <- h weights (cat idx 64:128), rows 64:128 <- feature
    wf = np.empty((7, 128, 9, 64), np.float32)
    for mm1 in range(7):
        for t in range(9):
            wf[mm1, 0:64, t] = fus_w[mm1, :, 64:128, t // 3, t % 3].T
            wf[mm1, 64:128, t] = fus_w[mm1, :, 0:64, t // 3, t % 3].T
    wf = wf.reshape(7 * 128, 576)
    wt = np.zeros((G, 128, 3, 6, 64), np.float32)
    for m in range(G):
        for cv, tw in enumerate((t2_w, t3_w, t4_w)):
            for dx in range(3):
                wt[m, 0:64, cv, dx] = tw[m, :, :, 0, dx].T
                wt[m, 64:128, cv, dx] = tw[m, :, :, 1, dx].T
                wt[m, 0:64, cv, 3 + dx] = tw[m, :, :, 2, dx].T
    wt = wt.reshape(G * 128, 1152)
    w5 = np.zeros((128, G * 6), np.float32)
    for m in range(G):
        for dx in range(3):
            w5[0:64, m * 6 + dx] = t5_w[m, 0, :, 0, dx]
            w5[64:128, m * 6 + dx] = t5_w[m, 0, :, 1, dx]
            w5[0:64, m * 6 + 3 + dx] = t5_w[m, 0, :, 2, dx]
    bb = np.zeros((64, 39), np.float32)
    aa = np.zeros((64, 39), np.float32)
    bb[:, 0:8] = h1_b.T; aa[:, 0:8] = np.broadcast_to(h1_a, (64, 8))
    bb[:, 8:15] = fus_b.T; aa[:, 8:15] = np.broadcast_to(fus_a, (64, 7))
    bb[:, 15:23] = t2_b.T; aa[:, 15:23] = np.broadcast_to(t2_a, (64, 8))
    bb[:, 23:31] = t3_b.T; aa[:, 23:31] = np.broadcast_to(t3_a, (64, 8))
    bb[:, 31:39] = t4_b.T; aa[:, 31:39] = np.broadcast_to(t4_a, (64, 8))

    in_maps = []
    for c in range(8):
        mmk = np.ones((128, 2), np.float32)
        if c == 0:
            mmk[:, 0] = 0.0
        if c == 7:
            mmk[:, 1] = 0.0
        in_maps.append({"r9": r9[c], "wh": wh, "wf": wf, "wt": wt, "w5": w5,
                        "bb": bb, "aa": aa, "mm": mmk})
    return in_maps


def kernel(x, sample_w, up_w, up_b, h1_w, h1_b, h1_a, fus_w, fus_b, fus_a,
           t2_w, t2_b, t2_a, t3_w, t3_b, t3_a, t4_w, t4_b, t4_a, t5_w, t5_b):
    in_maps = build_in_maps(
        x, sample_w, up_w, up_b, h1_w, h1_b, h1_a, fus_w, fus_b, fus_a,
        t2_w, t2_b, t2_a, t3_w, t3_b, t3_a, t4_w, t4_b, t4_a, t5_w, t5_b)
    results = _run(in_maps)
    out = np.empty((G, 1, 1, H, W), np.float32)
    for c in range(8):
        o = results[c]["o"].reshape(G, 32, 256)
        out[:, 0, 0, 32 * c:32 * c + 32, :] = o
    out += np.asarray(t5_b).reshape(G, 1, 1, 1, 1)
    return out



# revision 4
# speedup vs baseline: 132.6517x; 132.6517x over previous
"""Trainium2 Bass kernel for HierarchicalCSNet (8 groups, 256x256, G-fused chain).

Strategy: spatial row-sharding across 8 NeuronCores. Core c owns image rows
[32c, 32c+32) and recomputes shrinking halo margins locally (zero collectives).
The tiny head (strided sample conv + 1x1 upsample + block-scatter reshape) is
computed on host; everything from the first 3x3 conv onward runs on device.

v2: fp16 matmul operands (fp32 PSUM accumulate), PE array col-tiling so two
M=64 conv tiles run concurrently on the two column halves of the 128x128 PE
(tile_position (0,0)/(0,64)), 4-way row+col tiled K=9 h-convs, 4-way col-tiled
M=1 t5 convs, r9 head data preloaded to SBUF, all weights preloaded, and
2-tile-batched PReLU activations ([64,1024] ACTs spanning 2 PSUM banks).

Slot grid per core: slot s in [0,56) <-> global row 32c - 12 + s. All on-chip
feature rows are stored at pitch 258 (1 zero pad col each side). Margins:
feature_m needs margin M_m = 11 - m, h_m needs H_m = M_m + 1 (H_0 = 11).
Image-edge cores zero their out-of-image margin rows via per-core mask scalars.
Tail dy=2 taps use full-K=128 matmuls with zeroed upper-half weights so every
matmul in a conv keeps one PE tile mode.
"""
import sys, os
import numpy as np

for _p in ("/opt/trn_rl_repo", os.path.expanduser("~/.axon_site/_ro/trn_rl_repo")):
    if os.path.isdir(_p) and _p not in sys.path:
        sys.path.append(_p)

G, BS = 8, 32
H = W = 256
PITCH = 258
NSLOT = 54          # slots [1,55) stored in F/TA (row = slot-1)
TB_BASE = 9
TB_ROWS = 38        # slots [9,47) stored in TB (row = slot-9)
NBLK = 49           # r9 chunk blocks (4 chunks interleaved per block)


def _h_range(m):
    return (1, 55) if m == 0 else (m, 56 - m)


def _fus_range(m):
    return (m + 1, 55 - m)


def _chunks():
    """(m, s0) list for h-conv tiles, in program order."""
    out = []
    for m in range(G):
        lo, hi = _h_range(m)
        for s0 in range(lo, hi, 2):
            out.append((m, s0))
    return out


_CHUNKS = _chunks()
NCHUNK = len(_CHUNKS)   # 195

_BUILT = None


def _groups_of(lst):
    """Split into quads/pairs/singles (never 3)."""
    out, i = [], 0
    while i < len(lst):
        n = min(4, len(lst) - i)
        if n == 3:
            n = 2
        out.append(lst[i:i + n])
        i += n
    return out


def _build_program():
    import concourse.bacc as bacc
    import concourse.mybir as mybir
    import concourse.tile as tile

    f32 = mybir.dt.float32
    f16 = mybir.dt.float16
    PRELU = mybir.ActivationFunctionType.Prelu
    COPY = mybir.ActivationFunctionType.Copy

    nc = bacc.Bacc("TRN2", target_bir_lowering=False)
    r9_d = nc.dram_tensor("r9", [128, NBLK * 516], f16, kind="ExternalInput")
    wh_d = nc.dram_tensor("wh", [128, G * 64], f16, kind="ExternalInput")
    wf_d = nc.dram_tensor("wf", [128, 7 * 576], f16, kind="ExternalInput")
    wt_d = nc.dram_tensor("wt", [128, G * 1152], f16, kind="ExternalInput")
    w5_d = nc.dram_tensor("w5", [128, G * 6], f16, kind="ExternalInput")
    bb_d = nc.dram_tensor("bb", [128, 39], f32, kind="ExternalInput")
    aa_d = nc.dram_tensor("aa", [128, 39], f32, kind="ExternalInput")
    mm_d = nc.dram_tensor("mm", [128, 2], f32, kind="ExternalInput")
    o_d = nc.dram_tensor("o", [G, 32 * 256], f32, kind="ExternalOutput")

    with tile.TileContext(nc) as tc:
        with tc.tile_pool(name="const", bufs=1) as cst, \
             tc.tile_pool(name="big", bufs=1) as big, \
             tc.tile_pool(name="o5p", bufs=2) as o5p, \
             tc.tile_pool(name="ps", bufs=3, space="PSUM") as ps, \
             tc.tile_pool(name="p5", bufs=2, space="PSUM") as p5p:

            wh_t = cst.tile([128, G * 64], f16)
            wf_t = cst.tile([128, 7 * 576], f16)
            wt_t = cst.tile([128, G * 1152], f16)
            w5_t = cst.tile([128, G * 6], f16)
            bb_t = cst.tile([128, 39], f32)
            aa_t = cst.tile([128, 39], f32)
            mm_t = cst.tile([128, 2], f32)
            r9s = cst.tile([128, NBLK * 516], f16)
            F = big.tile([128, NSLOT * PITCH], f16)
            TA = big.tile([128, NSLOT * PITCH], f16)
            TB = big.tile([128, TB_ROWS * PITCH], f16)

            nc.sync.dma_start(wh_t[:], wh_d[:])
            nc.sync.dma_start(wf_t[:], wf_d[:])
            nc.sync.dma_start(wt_t[:], wt_d[:])
            nc.sync.dma_start(w5_t[:], w5_d[:])
            nc.sync.dma_start(bb_t[:], bb_d[:])
            nc.sync.dma_start(aa_t[:], aa_d[:])
            nc.sync.dma_start(mm_t[:], mm_d[:])
            for i in range(7):
                lo = i * 7 * 516
                hi = min(NBLK, (i + 1) * 7) * 516
                nc.sync.dma_start(r9s[:, lo:hi], r9_d[:, lo:hi])
            nc.vector.memset(F[:].bitcast(f32), 0.0)
            nc.vector.memset(TA[:].bitcast(f32), 0.0)
            nc.vector.memset(TB[:].bitcast(f32), 0.0)

            Fv = F[:].rearrange("p (r x) -> p r x", x=PITCH)
            TAv = TA[:].rearrange("p (r x) -> p r x", x=PITCH)
            TBv = TB[:].rearrange("p (r x) -> p r x", x=PITCH)
            r9v = r9s[:].rearrange("p (b r x) -> p b r x", r=2, x=258)

            # pre-zero the two rotating t5 PSUM buffers (the [128,512] copy
            # ACT reads partitions the M=1 matmuls never write)
            for _ in range(2):
                P5 = p5p.tile([128, 512], f32, tag="p5")
                nc.vector.memset(P5[:], 0.0)

            def mask(view, base, mlo, mhi, stacked=False, nrows=NSLOT):
                # zero out-of-image rows: top slots [mlo,12) with mm[:,0],
                # bottom slots [44,mhi) with mm[:,1] (no-op on interior cores)
                for (lo, hi, col) in ((mlo, 12, 0), (44, mhi, 1)):
                    if hi <= lo:
                        continue
                    nc.vector.tensor_scalar_mul(
                        view[0:64, lo - base:hi - base, :],
                        view[0:64, lo - base:hi - base, :],
                        mm_t[0:64, col:col + 1])
                if not stacked:
                    return
                # upper half holds rows shifted by +1 slot
                for (lo, hi, col) in ((mlo, 12, 0), (44, mhi, 1)):
                    rlo = max(0, lo - base - 1)
                    rhi = min(nrows, hi - base - 1)
                    if rhi <= rlo:
                        continue
                    nc.vector.tensor_scalar_mul(
                        view[64:128, rlo:rhi, :],
                        view[64:128, rlo:rhi, :],
                        mm_t[64:128, col:col + 1])

            def stack_quad(buf, base, w0, w1, nrows):
                # buf[64:128, r] := buf[0:64, r+1] for rows whose +1 slot lies
                # in the freshly written slot range [w0, w1)
                d0 = max(0, w0 - 1 - base)
                d1 = min(nrows - 1, w1 - 1 - base)
                if d1 <= d0:
                    return
                nc.sync.dma_start(
                    buf[64:128, d0 * PITCH:d1 * PITCH],
                    buf[0:64, (d0 + 1) * PITCH:(d1 + 1) * PITCH])

            def emit_conv(grp, ntaps, mm_fn, dst_v, dst_base, bcol,
                          dst_buf=None, dst_rows=None):
                """One group of 2-row tiles, col-paired on the PE array.

                grp: consecutive s0 values (len 4, 2, or 1). Tiles 0..1 go to
                PSUM partitions 0:64 (PE cols 0:64), tiles 2..3 to partitions
                64:128 (PE cols 64:128); the A/B chains run concurrently."""
                n = len(grp)
                P = ps.tile([128, 1024], f32, tag="ps")
                if n == 4:
                    reg = [(0, 0), (0, 512), (64, 0), (64, 512)]
                    order = [0, 2, 1, 3]
                elif n == 2:
                    reg = [(0, 0), (64, 0)]
                    order = [0, 1]
                else:
                    reg = [(0, 0)]
                    order = [0]
                for t in range(ntaps):
                    for ci in order:
                        part, c0 = reg[ci]
                        mm_fn(P[part:part + 64, c0:c0 + 512], t, grp[ci], part)
                if n == 4:
                    for part, s in ((0, grp[0]), (64, grp[2])):
                        nc.scalar.activation(
                            dst_v[0:64, s - dst_base:s - dst_base + 4, 1:257],
                            P[part:part + 64, 0:1024], PRELU,
                            bias=bb_t[part:part + 64, bcol:bcol + 1],
                            scale=1.0,
                            alpha=aa_t[part:part + 64, bcol:bcol + 1])
                else:
                    for ci in range(n):
                        part, c0 = reg[ci]
                        s = grp[ci]
                        nc.scalar.activation(
                            dst_v[0:64, s - dst_base:s - dst_base + 2, 1:257],
                            P[part:part + 64, c0:c0 + 512], PRELU,
                            bias=bb_t[part:part + 64, bcol:bcol + 1],
                            scale=1.0,
                            alpha=aa_t[part:part + 64, bcol:bcol + 1])
                if dst_buf is not None:
                    stack_quad(dst_buf, dst_base, grp[0], grp[-1] + 2, dst_rows)

            chunk_idx = 0
            for m in range(G):
                # --- h_m: K=9 matmuls from SBUF-resident r9, 4-way row-tiled
                h_lo, h_hi = _h_range(m)
                hdst = TAv if m == 0 else Fv
                for grp in _groups_of(list(range(h_lo, h_hi, 2))):
                    k0 = chunk_idx

                    def h_mm(region, t, s0, colpos, k0=k0, grp=grp, m=m):
                        k = k0 + grp.index(s0)
                        rb = 32 * (k % 4)
                        nc.tensor.matmul(
                            region, wh_t[rb:rb + 9, m * 64:(m + 1) * 64],
                            r9v[rb:rb + 9, k // 4, 0:2, 1:257],
                            start=True, stop=True,
                            tile_position=(rb, colpos))

                    emit_conv(grp, 1, h_mm, hdst, 1, m,
                              TA if m == 0 else None,
                              NSLOT if m == 0 else None)
                    chunk_idx += len(grp)
                mask(hdst, 1, h_lo, h_hi, stacked=(m == 0))

                # --- fusion m (m>=1): 9 taps K=128 from F = [h_m | feat_{m-1}]
                if m >= 1:
                    f_lo, f_hi = _fus_range(m)
                    for grp in _groups_of(list(range(f_lo, f_hi, 2))):

                        def fus_mm(region, t, s0, colpos, m=m):
                            dy, dx = t // 3, t % 3
                            rr = s0 + dy - 2
                            w0 = (m - 1) * 576 + t * 64
                            nc.tensor.matmul(
                                region, wf_t[:, w0:w0 + 64],
                                Fv[0:128, rr:rr + 2, dx:dx + 256],
                                start=(t == 0), stop=(t == 8))

                        emit_conv(grp, 9, fus_mm, TAv, 1, 8 + m - 1, TA, NSLOT)
                    mask(TAv, 1, f_lo, f_hi, stacked=True)

                # --- feature_m (in TA) -> F[64:128] for next fusion ---
                if m < G - 1:
                    lo, hi = (1, 55) if m == 0 else _fus_range(m)
                    nc.sync.dma_start(
                        F[64:128, (lo - 1) * PITCH:(hi - 1) * PITCH],
                        TA[0:64, (lo - 1) * PITCH:(hi - 1) * PITCH])

                # --- tails (dy-packed: 3x K=128 + 3x padded-K=128 per tile) ---
                def tconv(src_v, src_base, dst_v, dst_base, dst_buf, dst_rows,
                          lo, hi, cv, bcol, m=m):
                    for grp in _groups_of(list(range(lo, hi, 2))):

                        def t_mm(region, j, s0, colpos):
                            dx = j % 3
                            c0 = m * 1152 + (cv * 6 + j) * 64
                            rr = (s0 - 1 - src_base) if j < 3 \
                                else (s0 + 1 - src_base)
                            nc.tensor.matmul(
                                region, wt_t[:, c0:c0 + 64],
                                src_v[0:128, rr:rr + 2, dx:dx + 256],
                                start=(j == 0), stop=(j == 5))

                        emit_conv(grp, 6, t_mm, dst_v, dst_base, bcol,
                                  dst_buf, dst_rows)
                    mask(dst_v, dst_base, lo, hi, stacked=True, nrows=dst_rows)

                tconv(TAv, 1, TBv, TB_BASE, TB, TB_ROWS, 9, 47, 0, 15 + m)
                tconv(TBv, TB_BASE, TAv, 1, TA, NSLOT, 10, 46, 1, 23 + m)
                tconv(TAv, 1, TBv, TB_BASE, TB, TB_ROWS, 11, 45, 2, 31 + m)

                # --- t5: M=1, 4-way col-tiled (4 tiles per PSUM bank) ---
                for q in range(4):
                    s0 = 12 + 8 * q
                    P5 = p5p.tile([128, 512], f32, tag="p5")
                    for j in range(6):
                        dx = j % 3
                        c5 = m * 6 + j
                        for ti in range(4):
                            sT = s0 + 2 * ti
                            rr = (sT - 1 - TB_BASE) if j < 3 \
                                else (sT + 1 - TB_BASE)
                            nc.tensor.matmul(
                                P5[32 * ti:32 * ti + 1, :],
                                w5_t[:, c5:c5 + 1],
                                TBv[0:128, rr:rr + 2, dx:dx + 256],
                                start=(j == 0), stop=(j == 5),
                                tile_position=(0, 32 * ti))
                    o5 = o5p.tile([128, 512], f32, tag="o5")
                    nc.scalar.activation(o5[:], P5[:], COPY)
                    for ti in range(4):
                        sT = s0 + 2 * ti
                        nc.sync.dma_start(
                            o_d[m, (sT - 12) * 256:(sT - 10) * 256],
                            o5[32 * ti:32 * ti + 1, :])

    nc.compile()
    return nc


def _get_program():
    global _BUILT
    if _BUILT is None:
        _BUILT = _build_program()
    return _BUILT


def _host_heads(x, sample_w, up_w, up_b):
    """r[m] (256x256) for all groups, float32."""
    X = x[0, 0].reshape(8, 32, 8, 32).astype(np.float64)
    R = np.empty((G, H, W), np.float32)
    for m in range(G):
        S = np.einsum('ipjq,cpq->cij', X, sample_w[m, :, 0].astype(np.float64))
        U = np.einsum('cij,uc->uij', S, up_w[m, :, :, 0, 0].astype(np.float64))
        U = U + up_b[m].astype(np.float64)[:, None, None]
        R[m] = U.reshape(32, 32, 8, 8).transpose(2, 0, 3, 1).reshape(256, 256)
    return R


def _build_r9(R):
    """Per-core h-conv rhs, fp16. Chunk k (program order) tap t lives at
    partition 32*(k%4)+t, block k//4 (516 cols: 2 slot rows, pitch 258)."""
    from numpy.lib.stride_tricks import sliding_window_view
    rp = np.zeros((G, H + 26, W + 4), np.float32)   # rows g+13, cols x+2
    rp[:, 13:13 + H, 2:2 + W] = R
    tmp = np.empty((8, NCHUNK, 9, 516), np.float32)
    k0 = 0
    for m in range(G):
        lo, hi = _h_range(m)
        s0s = np.arange(lo, hi, 2)
        SW = sliding_window_view(rp[m], (2, 258))
        for t in range(9):
            dy, dx = t // 3, t % 3
            g0 = (32 * np.arange(8))[:, None] + s0s[None, :] + dy
            tmp[:, k0:k0 + len(s0s), t] = SW[g0, dx].reshape(8, len(s0s), 516)
        k0 += len(s0s)
    out = np.zeros((8, 128, NBLK, 516), np.float16)
    for r in range(4):
        ks = np.arange(r, NCHUNK, 4)
        out[:, 32 * r:32 * r + 9, :len(ks)] = \
            tmp[:, ks].transpose(0, 2, 1, 3).astype(np.float16)
    return out.reshape(8, 128, NBLK * 516)


_EXEC = None


def _get_executor():
    """Persistent jitted shard_map executor over 8 cores (mirrors
    bass2jax.run_bass_via_pjrt, but reusable for repeat timing)."""
    global _EXEC
    if _EXEC is not None:
        return _EXEC
    import jax
    from jax.sharding import Mesh, PartitionSpec
    from jax.experimental.shard_map import shard_map
    import concourse.mybir as mybir
    from concourse import bass2jax

    nc = _get_program()
    bass2jax.install_neuronx_cc_hook()

    part_name = nc.partition_id_tensor.name if nc.partition_id_tensor else None
    in_names, out_names, out_avals, zero_shapes = [], [], [], []
    for alloc in nc.m.functions[0].allocations:
        if not isinstance(alloc, mybir.MemoryLocationSet):
            continue
        name = alloc.memorylocations[0].name
        if alloc.kind == "ExternalInput":
            if name != part_name:
                in_names.append(name)
        elif alloc.kind == "ExternalOutput":
            out_names.append(name)
            shape = tuple(alloc.tensor_shape)
            dtype = mybir.dt.np(alloc.dtype)
            out_avals.append(jax.core.ShapedArray(shape, dtype))
            zero_shapes.append((shape, dtype))
    n_params = len(in_names)
    all_names = in_names + out_names
    if part_name is not None:
        all_names = all_names + [part_name]

    def _body(*args):
        operands = list(args)
        if part_name is not None:
            operands.append(bass2jax.partition_id_tensor())
        outs = bass2jax._bass_exec_p.bind(
            *operands,
            out_avals=tuple(out_avals),
            in_names=tuple(all_names),
            out_names=tuple(out_names),
            lowering_input_output_aliases=(),
            sim_require_finite=True,
            sim_require_nnan=True,
            nc=nc,
        )
        return tuple(outs)

    devices = jax.devices()[:8]
    mesh = Mesh(np.asarray(devices), ("core",))
    n_outs = len(out_names)
    sharded = jax.jit(
        shard_map(_body, mesh=mesh,
                  in_specs=(PartitionSpec("core"),) * (n_params + n_outs),
                  out_specs=(PartitionSpec("core"),) * n_outs,
                  check_rep=False),
        keep_unused=True)
    _EXEC = (sharded, in_names, out_names, zero_shapes)
    return _EXEC


def _prep_device_args(in_maps):
    import jax
    sharded, in_names, out_names, zero_shapes = _get_executor()
    concat_in = [np.concatenate([in_maps[c][n] for c in range(8)], axis=0)
                 for n in in_names]
    concat_zero = [np.zeros((8 * s[0],) + tuple(s[1:]), d)
                   for (s, d) in zero_shapes]
    return [jax.device_put(a) for a in concat_in + concat_zero]


def _run(in_maps):
    sharded, in_names, out_names, zero_shapes = _get_executor()
    args = _prep_device_args(in_maps)
    outs = sharded(*args)
    res = []
    for c in range(8):
        res.append({n: np.asarray(outs[i]).reshape((8,) + zero_shapes[i][0])[c]
                    for i, n in enumerate(out_names)})
    return res


def bench(in_maps, iters=5):
    """Device-resident repeat timing of the sharded program (wall-clock,
    dominated by the fixed axon-RPC dispatch). Returns (best_s, times)."""
    import time as _t
    sharded, *_ = _get_executor()
    args = _prep_device_args(in_maps)
    r = sharded(*args)
    [x.block_until_ready() for x in r]
    times = []
    for _ in range(iters):
        t0 = _t.perf_counter()
        r = sharded(*args)
        [x.block_until_ready() for x in r]
        times.append(_t.perf_counter() - t0)
    return min(times), times


def build_in_maps(x, sample_w, up_w, up_b, h1_w, h1_b, h1_a, fus_w, fus_b,
                  fus_a, t2_w, t2_b, t2_a, t3_w, t3_b, t3_a, t4_w, t4_b,
                  t4_a, t5_w, t5_b):

    R = _host_heads(x, sample_w, up_w, up_b)
    r9 = _build_r9(R)

    # h weights: tap rows replicated at partition bases 0/32/64/96
    wh9 = h1_w[:, :, 0].reshape(G, 64, 9).transpose(2, 0, 1).reshape(9, G * 64)
    wh = np.zeros((128, G * 64), np.float16)
    for r in range(4):
        wh[32 * r:32 * r + 9] = wh9
    # fusion lhsT rows 0:64 <- h weights (cat idx 64:128), rows 64:128 <- feat
    wf = np.empty((128, 7, 9, 64), np.float32)
    for mm1 in range(7):
        for t in range(9):
            wf[0:64, mm1, t] = fus_w[mm1, :, 64:128, t // 3, t % 3].T
            wf[64:128, mm1, t] = fus_w[mm1, :, 0:64, t // 3, t % 3].T
    wf = wf.reshape(128, 7 * 576).astype(np.float16)
    wt = np.zeros((128, G, 3, 6, 64), np.float32)
    for m in range(G):
        for cv, tw in enumerate((t2_w, t3_w, t4_w)):
            for dx in range(3):
                wt[0:64, m, cv, dx] = tw[m, :, :, 0, dx].T
                wt[64:128, m, cv, dx] = tw[m, :, :, 1, dx].T
                wt[0:64, m, cv, 3 + dx] = tw[m, :, :, 2, dx].T
    wt = wt.reshape(128, G * 1152).astype(np.float16)
    w5 = np.zeros((128, G * 6), np.float32)
    for m in range(G):
        for dx in range(3):
            w5[0:64, m * 6 + dx] = t5_w[m, 0, :, 0, dx]
            w5[64:128, m * 6 + dx] = t5_w[m, 0, :, 1, dx]
            w5[0:64, m * 6 + 3 + dx] = t5_w[m, 0, :, 2, dx]
    w5 = w5.astype(np.float16)
    bb = np.zeros((64, 39), np.float32)
    aa = np.zeros((64, 39), np.float32)
    bb[:, 0:8] = h1_b.T; aa[:, 0:8] = np.broadcast_to(h1_a, (64, 8))
    bb[:, 8:15] = fus_b.T; aa[:, 8:15] = np.broadcast_to(fus_a, (64, 7))
    bb[:, 15:23] = t2_b.T; aa[:, 15:23] = np.broadcast_to(t2_a, (64, 8))
    bb[:, 23:31] = t3_b.T; aa[:, 23:31] = np.broadcast_to(t3_a, (64, 8))
    bb[:, 31:39] = t4_b.T; aa[:, 31:39] = np.broadcast_to(t4_a, (64, 8))
    bb = np.concatenate([bb, bb], axis=0)   # B-chain ACTs read rows 64:128
    aa = np.concatenate([aa, aa], axis=0)

    in_maps = []
    for c in range(8):
        mmk = np.ones((128, 2), np.float32)
        if c == 0:
            mmk[:, 0] = 0.0
        if c == 7:
            mmk[:, 1] = 0.0
        in_maps.append({"r9": r9[c], "wh": wh, "wf": wf, "wt": wt, "w5": w5,
                        "bb": bb, "aa": aa, "mm": mmk})
    return in_maps


def kernel(x, sample_w, up_w, up_b, h1_w, h1_b, h1_a, fus_w, fus_b, fus_a,
           t2_w, t2_b, t2_a, t3_w, t3_b, t3_a, t4_w, t4_b, t4_a, t5_w, t5_b):
    in_maps = build_in_maps(
        x, sample_w, up_w, up_b, h1_w, h1_b, h1_a, fus_w, fus_b, fus_a,
        t2_w, t2_b, t2_a, t3_w, t3_b, t3_a, t4_w, t4_b, t4_a, t5_w, t5_b)
    results = _run(in_maps)
    out = np.empty((G, 1, 1, H, W), np.float32)
    for c in range(8):
        o = results[c]["o"].reshape(G, 32, 256)
        out[:, 0, 0, 32 * c:32 * c + 32, :] = o
    out += np.asarray(t5_b).reshape(G, 1, 1, 1, 1)
    return out


# revision 26
# speedup vs baseline: 153.6308x; 1.1582x over previous
"""Trainium2 Bass kernel for HierarchicalCSNet (8 groups, 256x256, G-fused chain).

Strategy: spatial row-sharding across 8 NeuronCores. Core c owns image rows
[32c, 32c+32) and recomputes shrinking halo margins locally (zero collectives).
The tiny head (strided sample conv + 1x1 upsample + block-scatter reshape) is
computed on host; everything from the first 3x3 conv onward runs on device.

v4: fp16 matmul operands (fp32 PSUM accumulate), PE array col-tiling so two
M=64 conv tiles run concurrently on the two column halves of the 128x128 PE
(tile_position (0,0)/(0,64)), 4-way col-tiled M=1 t5 convs, all weights
preloaded, 2-tile-batched PReLU activations ([64,1024] ACTs spanning 2 PSUM
banks), and the tiny K=9 h-convs folded into the host head (their feature
slabs stream in via DMA, overlapped with the previous group's tails) so the
ACT engine stays below the PE roofline.

Slot grid per core: slot s in [0,56) <-> global row 32c - 12 + s. All on-chip
feature rows are stored at pitch 258 (1 zero pad col each side). Margins:
feature_m needs margin M_m = 11 - m, h_m needs H_m = M_m + 1 (H_0 = 11).
Image-edge cores zero their out-of-image margin rows via per-core mask scalars.
Tail dy=2 taps use full-K=128 matmuls with zeroed upper-half weights so every
matmul in a conv keeps one PE tile mode.
"""
import sys, os
import numpy as np

for _p in ("/opt/trn_rl_repo", os.path.expanduser("~/.axon_site/_ro/trn_rl_repo")):
    if os.path.isdir(_p) and _p not in sys.path:
        sys.path.append(_p)

G, BS = 8, 32
H = W = 256
PITCH = 258
NSLOT = 54          # slots [1,55) stored in F/TA (row = slot-1)
TB_BASE = 9
TB_ROWS = 38        # slots [9,47) stored in TB (row = slot-9)


def _fus_range(m):
    return (m + 1, 55 - m)


_BUILT = None


def _groups_of(lst):
    """Split into quads/pairs/singles (never 3)."""
    out, i = [], 0
    while i < len(lst):
        n = min(4, len(lst) - i)
        if n == 3:
            n = 2
        out.append(lst[i:i + n])
        i += n
    return out


def _build_program():
    import concourse.bacc as bacc
    import concourse.mybir as mybir
    import concourse.tile as tile

    f32 = mybir.dt.float32
    f16 = mybir.dt.float16
    PRELU = mybir.ActivationFunctionType.Prelu
    COPY = mybir.ActivationFunctionType.Copy

    SL = NSLOT * PITCH
    nc = bacc.Bacc("TRN2", target_bir_lowering=False)
    ha_d = nc.dram_tensor("ha", [128, SL], f16, kind="ExternalInput")
    fi_d = nc.dram_tensor("fi", [128, SL], f16, kind="ExternalInput")
    hf_d = nc.dram_tensor("hf", [64, 7 * SL], f16, kind="ExternalInput")
    wf_d = nc.dram_tensor("wf", [128, 7 * 576], f16, kind="ExternalInput")
    wt_d = nc.dram_tensor("wt", [128, G * 1152], f16, kind="ExternalInput")
    w5_d = nc.dram_tensor("w5", [128, G * 6], f16, kind="ExternalInput")
    bb_d = nc.dram_tensor("bb", [128, 39], f32, kind="ExternalInput")
    aa_d = nc.dram_tensor("aa", [128, 39], f32, kind="ExternalInput")
    mm_d = nc.dram_tensor("mm", [128, 2], f32, kind="ExternalInput")
    o_d = nc.dram_tensor("o", [G, 32 * 256], f32, kind="ExternalOutput")

    with tile.TileContext(nc) as tc:
        with tc.tile_pool(name="const", bufs=1) as cst, \
             tc.tile_pool(name="big", bufs=1) as big, \
             tc.tile_pool(name="o5p", bufs=2) as o5p, \
             tc.tile_pool(name="ps", bufs=3, space="PSUM") as ps, \
             tc.tile_pool(name="p5", bufs=2, space="PSUM") as p5p:

            wf_t = cst.tile([128, 7 * 576], f16)
            wt_t = cst.tile([128, G * 1152], f16)
            w5_t = cst.tile([128, G * 6], f16)
            bb_t = cst.tile([128, 39], f32)
            aa_t = cst.tile([128, 39], f32)
            mm_t = cst.tile([128, 2], f32)
            F = big.tile([128, NSLOT * PITCH], f16)
            TA = big.tile([128, NSLOT * PITCH], f16)
            TB = big.tile([128, TB_ROWS * PITCH], f16)

            # startup loads ordered so t2(0) [wt slab 0 + first TA rows] and
            # fus(1) [wf + host-prebuilt F] unblock as early as possible
            nc.sync.dma_start(mm_t[:], mm_d[:])
            nc.sync.dma_start(bb_t[:], bb_d[:])
            nc.sync.dma_start(aa_t[:], aa_d[:])
            nc.sync.dma_start(w5_t[:], w5_d[:])
            nc.sync.dma_start(wt_t[:, 0:1152], wt_d[:, 0:1152])
            for a, b in ((0, 18), (18, 36), (36, NSLOT)):
                nc.sync.dma_start(TA[:, a * PITCH:b * PITCH],
                                  ha_d[:, a * PITCH:b * PITCH])
            nc.sync.dma_start(wf_t[:], wf_d[:])
            # F preloaded whole: lower = h_1 slab, upper = feature_0
            nc.sync.dma_start(F[:], fi_d[:])
            for m_ in range(1, G):
                nc.sync.dma_start(wt_t[:, m_ * 1152:(m_ + 1) * 1152],
                                  wt_d[:, m_ * 1152:(m_ + 1) * 1152])
            nc.vector.memset(TB[:].bitcast(f32), 0.0)

            Fv = F[:].rearrange("p (r x) -> p r x", x=PITCH)
            TAv = TA[:].rearrange("p (r x) -> p r x", x=PITCH)
            TBv = TB[:].rearrange("p (r x) -> p r x", x=PITCH)

            # pre-zero the two rotating t5 PSUM buffers (the [128,512] copy
            # ACT reads partitions the M=1 matmuls never write)
            for _ in range(2):
                P5 = p5p.tile([128, 512], f32, tag="p5")
                nc.vector.memset(P5[:], 0.0)

            def mask(view, base, mlo, mhi, stacked=False, nrows=NSLOT):
                # zero out-of-image rows: top slots [mlo,12) with mm[:,0],
                # bottom slots [44,mhi) with mm[:,1] (no-op on interior cores)
                for (lo, hi, col) in ((mlo, 12, 0), (44, mhi, 1)):
                    if hi <= lo:
                        continue
                    nc.vector.tensor_scalar_mul(
                        view[0:64, lo - base:hi - base, :],
                        view[0:64, lo - base:hi - base, :],
                        mm_t[0:64, col:col + 1])
                if not stacked:
                    return
                # upper half holds rows shifted by +1 slot
                for (lo, hi, col) in ((mlo, 12, 0), (44, mhi, 1)):
                    rlo = max(0, lo - base - 1)
                    rhi = min(nrows, hi - base - 1)
                    if rhi <= rlo:
                        continue
                    nc.vector.tensor_scalar_mul(
                        view[64:128, rlo:rhi, :],
                        view[64:128, rlo:rhi, :],
                        mm_t[64:128, col:col + 1])

            def stack_quad(buf, base, w0, w1, nrows):
                # buf[64:128, r] := buf[0:64, r+1] for rows whose +1 slot lies
                # in the freshly written slot range [w0, w1)
                d0 = max(0, w0 - 1 - base)
                d1 = min(nrows - 1, w1 - 1 - base)
                if d1 <= d0:
                    return
                nc.sync.dma_start(
                    buf[64:128, d0 * PITCH:d1 * PITCH],
                    buf[0:64, (d0 + 1) * PITCH:(d1 + 1) * PITCH])

            def emit_conv(grp, ntaps, mm_fn, dst_v, dst_base, bcol,
                          dst_buf=None, dst_rows=None):
                """One group of 2-row tiles, col-paired on the PE array.

                grp: consecutive s0 values (len 4, 2, or 1). Tiles 0..1 go to
                PSUM partitions 0:64 (PE cols 0:64), tiles 2..3 to partitions
                64:128 (PE cols 64:128); the A/B chains run concurrently."""
                n = len(grp)
                P = ps.tile([128, 1024], f32, tag="ps")
                if n == 4:
                    reg = [(0, 0), (0, 512), (64, 0), (64, 512)]
                    order = [0, 2, 1, 3]
                elif n == 2:
                    reg = [(0, 0), (64, 0)]
                    order = [0, 1]
                else:
                    reg = [(0, 0)]
                    order = [0]
                for t in range(ntaps):
                    for ci in order:
                        part, c0 = reg[ci]
                        mm_fn(P[part:part + 64, c0:c0 + 512], t, grp[ci], part)
                if n == 4:
                    for part, s in ((0, grp[0]), (64, grp[2])):
                        nc.scalar.activation(
                            dst_v[0:64, s - dst_base:s - dst_base + 4, 1:257],
                            P[part:part + 64, 0:1024], PRELU,
                            bias=bb_t[part:part + 64, bcol:bcol + 1],
                            scale=1.0,
                            alpha=aa_t[part:part + 64, bcol:bcol + 1])
                else:
                    for ci in range(n):
                        part, c0 = reg[ci]
                        s = grp[ci]
                        nc.scalar.activation(
                            dst_v[0:64, s - dst_base:s - dst_base + 2, 1:257],
                            P[part:part + 64, c0:c0 + 512], PRELU,
                            bias=bb_t[part:part + 64, bcol:bcol + 1],
                            scale=1.0,
                            alpha=aa_t[part:part + 64, bcol:bcol + 1])
                if dst_buf is not None:
                    stack_quad(dst_buf, dst_base, grp[0], grp[-1] + 2, dst_rows)

            def load_h(m):
                # host-computed h_m feature slab -> F lower (fusion rhs).
                nc.sync.dma_start(F[0:64, :], hf_d[:, (m - 1) * SL:m * SL])

            def copy_feature(m):
                # feature_m (in TA) -> F[64:128] for the next fusion
                lo, hi = (1, 55) if m == 0 else _fus_range(m)
                nc.sync.dma_start(
                    F[64:128, (lo - 1) * PITCH:(hi - 1) * PITCH],
                    TA[0:64, (lo - 1) * PITCH:(hi - 1) * PITCH])

            for m in range(G):
                # --- fusion m (m>=1): 9 taps K=128 from F = [h_m | feat_{m-1}]
                if m >= 1:
                    f_lo, f_hi = _fus_range(m)
                    for grp in _groups_of(list(range(f_lo, f_hi, 2))):

                        def fus_mm(region, t, s0, colpos, m=m):
                            dy, dx = t // 3, t % 3
                            rr = s0 + dy - 2
                            w0 = (m - 1) * 576 + t * 64
                            nc.tensor.matmul(
                                region, wf_t[:, w0:w0 + 64],
                                Fv[0:128, rr:rr + 2, dx:dx + 256],
                                start=(t == 0), stop=(t == 8))

                        emit_conv(grp, 9, fus_mm, TAv, 1, 8 + m - 1, TA, NSLOT)
                    mask(TAv, 1, f_lo, f_hi, stacked=True)
                    if m < G - 1:
                        copy_feature(m)
                        load_h(m + 1)

                # --- tails (dy-packed: 3x K=128 + 3x padded-K=128 per tile) ---
                def tconv(src_v, src_base, dst_v, dst_base, dst_buf, dst_rows,
                          lo, hi, cv, bcol, m=m):
                    for grp in _groups_of(list(range(lo, hi, 2))):

                        def t_mm(region, j, s0, colpos):
                            dx = j % 3
                            c0 = m * 1152 + (cv * 6 + j) * 64
                            rr = (s0 - 1 - src_base) if j < 3 \
                                else (s0 + 1 - src_base)
                            nc.tensor.matmul(
                                region, wt_t[:, c0:c0 + 64],
                                src_v[0:128, rr:rr + 2, dx:dx + 256],
                                start=(j == 0), stop=(j == 5))

                        emit_conv(grp, 6, t_mm, dst_v, dst_base, bcol,
                                  dst_buf, dst_rows)
                    mask(dst_v, dst_base, lo, hi, stacked=True, nrows=dst_rows)

                tconv(TAv, 1, TBv, TB_BASE, TB, TB_ROWS, 9, 47, 0, 15 + m)
                tconv(TBv, TB_BASE, TAv, 1, TA, NSLOT, 10, 46, 1, 23 + m)
                tconv(TAv, 1, TBv, TB_BASE, TB, TB_ROWS, 11, 45, 2, 31 + m)

                # --- t5: M=1, 4-way col-tiled (4 tiles per PSUM bank) ---
                for q in range(4):
                    s0 = 12 + 8 * q
                    P5 = p5p.tile([128, 512], f32, tag="p5")
                    for j in range(6):
                        dx = j % 3
                        c5 = m * 6 + j
                        for ti in range(4):
                            sT = s0 + 2 * ti
                            rr = (sT - 1 - TB_BASE) if j < 3 \
                                else (sT + 1 - TB_BASE)
                            nc.tensor.matmul(
                                P5[32 * ti:32 * ti + 1, :],
                                w5_t[:, c5:c5 + 1],
                                TBv[0:128, rr:rr + 2, dx:dx + 256],
                                start=(j == 0), stop=(j == 5),
                                tile_position=(0, 32 * ti))
                    o5 = o5p.tile([128, 512], f32, tag="o5")
                    if m == G - 1:
                        nc.vector.tensor_scalar_add(o5[:], P5[:], 0.0)
                    else:
                        nc.scalar.activation(o5[:], P5[:], COPY)
                    for ti in range(4):
                        sT = s0 + 2 * ti
                        nc.sync.dma_start(
                            o_d[m, (sT - 12) * 256:(sT - 10) * 256],
                            o5[32 * ti:32 * ti + 1, :])

    nc.compile()
    return nc


def _get_program():
    global _BUILT
    if _BUILT is None:
        _BUILT = _build_program()
    return _BUILT


def _host_heads(x, sample_w, up_w, up_b):
    """r[m] (256x256) for all groups, float32."""
    X = x[0, 0].reshape(8, 32, 8, 32).astype(np.float64)
    R = np.empty((G, H, W), np.float32)
    for m in range(G):
        S = np.einsum('ipjq,cpq->cij', X, sample_w[m, :, 0].astype(np.float64))
        U = np.einsum('cij,uc->uij', S, up_w[m, :, :, 0, 0].astype(np.float64))
        U = U + up_b[m].astype(np.float64)[:, None, None]
        R[m] = U.reshape(32, 32, 8, 8).transpose(2, 0, 3, 1).reshape(256, 256)
    return R


def _host_h(R, h1_w, h1_b, h1_a):
    """Full-image h_m = PReLU(conv3x3(r_m) + b), [G, 64, 256*256] float32.
    0.6 GFLOP on host (0.3% of the net) — frees 138us of device ACT time."""
    Rp = np.zeros((G, 258, 258), np.float32)
    Rp[:, 1:257, 1:257] = R
    Rs = np.empty((G, 9, 256 * 256), np.float32)
    for t in range(9):
        dy, dx = t // 3, t % 3
        Rs[:, t] = Rp[:, dy:dy + 256, dx:dx + 256].reshape(G, -1)
    w9 = h1_w[:, :, 0].reshape(G, 64, 9).astype(np.float32)
    Hf = np.empty((G, 64, 256 * 256), np.float32)
    for g in range(G):
        Hf[g] = w9[g] @ Rs[g]
    Hf += h1_b.astype(np.float32)[:, :, None]
    a = h1_a.astype(np.float32)[:, None, None]
    Hf = np.where(Hf >= 0, Hf, a * Hf)
    return Hf.reshape(G, 64, 256, 256)


def _build_h_slabs(Hf):
    """Per-core slabs: ha [8,128,SL] (h_0 into TA: stacked upper, edge rows
    zeroed), hf [8,64,7*SL] (h_1..h_7 into F lower). Slot r+1 <-> global row
    32c-11+r; out-of-image rows stay zero (replaces the device masks)."""
    SL = NSLOT * PITCH
    ha = np.zeros((8, 128, NSLOT, PITCH), np.float16)
    hfd = np.zeros((8, 64, 7, NSLOT, PITCH), np.float16)
    for c in range(8):
        gr = 32 * c - 11 + np.arange(NSLOT)
        val = (gr >= 0) & (gr < 256)
        rows = gr[val]
        ha[c][0:64, val, 1:257] = Hf[0][:, rows].astype(np.float16)
        for m in range(1, 8):
            hfd[c, :, m - 1][:, val, 1:257] = \
                Hf[m][:, rows].astype(np.float16)
    ha[:, 64:128, 0:NSLOT - 1] = ha[:, 0:64, 1:NSLOT]
    # initial F tile: lower = h_1 slab, upper = feature_0 (= h_0)
    fi = np.zeros((8, 128, NSLOT, PITCH), np.float16)
    fi[:, 0:64] = hfd[:, :, 0]
    fi[:, 64:128] = ha[:, 0:64]
    return (ha.reshape(8, 128, SL), fi.reshape(8, 128, SL),
            hfd.reshape(8, 64, 7 * SL))


_EXEC = None


def _get_executor():
    """Persistent jitted shard_map executor over 8 cores (mirrors
    bass2jax.run_bass_via_pjrt, but reusable for repeat timing)."""
    global _EXEC
    if _EXEC is not None:
        return _EXEC
    import jax
    from jax.sharding import Mesh, PartitionSpec
    from jax.experimental.shard_map import shard_map
    import concourse.mybir as mybir
    from concourse import bass2jax

    nc = _get_program()
    bass2jax.install_neuronx_cc_hook()

    part_name = nc.partition_id_tensor.name if nc.partition_id_tensor else None
    in_names, out_names, out_avals, zero_shapes = [], [], [], []
    for alloc in nc.m.functions[0].allocations:
        if not isinstance(alloc, mybir.MemoryLocationSet):
            continue
        name = alloc.memorylocations[0].name
        if alloc.kind == "ExternalInput":
            if name != part_name:
                in_names.append(name)
        elif alloc.kind == "ExternalOutput":
            out_names.append(name)
            shape = tuple(alloc.tensor_shape)
            dtype = mybir.dt.np(alloc.dtype)
            out_avals.append(jax.core.ShapedArray(shape, dtype))
            zero_shapes.append((shape, dtype))
    n_params = len(in_names)
    all_names = in_names + out_names
    if part_name is not None:
        all_names = all_names + [part_name]

    def _body(*args):
        operands = list(args)
        if part_name is not None:
            operands.append(bass2jax.partition_id_tensor())
        outs = bass2jax._bass_exec_p.bind(
            *operands,
            out_avals=tuple(out_avals),
            in_names=tuple(all_names),
            out_names=tuple(out_names),
            lowering_input_output_aliases=(),
            sim_require_finite=True,
            sim_require_nnan=True,
            nc=nc,
        )
        return tuple(outs)

    devices = jax.devices()[:8]
    mesh = Mesh(np.asarray(devices), ("core",))
    n_outs = len(out_names)
    sharded = jax.jit(
        shard_map(_body, mesh=mesh,
                  in_specs=(PartitionSpec("core"),) * (n_params + n_outs),
                  out_specs=(PartitionSpec("core"),) * n_outs,
                  check_rep=False),
        keep_unused=True)
    _EXEC = (sharded, in_names, out_names, zero_shapes)
    return _EXEC


def _prep_device_args(in_maps):
    import jax
    sharded, in_names, out_names, zero_shapes = _get_executor()
    concat_in = [np.concatenate([in_maps[c][n] for c in range(8)], axis=0)
                 for n in in_names]
    concat_zero = [np.zeros((8 * s[0],) + tuple(s[1:]), d)
                   for (s, d) in zero_shapes]
    return [jax.device_put(a) for a in concat_in + concat_zero]


def _run(in_maps):
    sharded, in_names, out_names, zero_shapes = _get_executor()
    args = _prep_device_args(in_maps)
    outs = sharded(*args)
    res = []
    for c in range(8):
        res.append({n: np.asarray(outs[i]).reshape((8,) + zero_shapes[i][0])[c]
                    for i, n in enumerate(out_names)})
    return res


def bench(in_maps, iters=5):
    """Device-resident repeat timing of the sharded program (wall-clock,
    dominated by the fixed axon-RPC dispatch). Returns (best_s, times)."""
    import time as _t
    sharded, *_ = _get_executor()
    args = _prep_device_args(in_maps)
    r = sharded(*args)
    [x.block_until_ready() for x in r]
    times = []
    for _ in range(iters):
        t0 = _t.perf_counter()
        r = sharded(*args)
        [x.block_until_ready() for x in r]
        times.append(_t.perf_counter() - t0)
    return min(times), times


def build_in_maps(x, sample_w, up_w, up_b, h1_w, h1_b, h1_a, fus_w, fus_b,
                  fus_a, t2_w, t2_b, t2_a, t3_w, t3_b, t3_a, t4_w, t4_b,
                  t4_a, t5_w, t5_b):

    R = _host_heads(x, sample_w, up_w, up_b)
    Hf = _host_h(R, np.asarray(h1_w), np.asarray(h1_b), np.asarray(h1_a))
    ha, fi, hfd = _build_h_slabs(Hf)

    # fusion lhsT rows 0:64 <- h weights (cat idx 64:128), rows 64:128 <- feat
    wf = np.empty((128, 7, 9, 64), np.float32)
    for mm1 in range(7):
        for t in range(9):
            wf[0:64, mm1, t] = fus_w[mm1, :, 64:128, t // 3, t % 3].T
            wf[64:128, mm1, t] = fus_w[mm1, :, 0:64, t // 3, t % 3].T
    wf = wf.reshape(128, 7 * 576).astype(np.float16)
    wt = np.zeros((128, G, 3, 6, 64), np.float32)
    for m in range(G):
        for cv, tw in enumerate((t2_w, t3_w, t4_w)):
            for dx in range(3):
                wt[0:64, m, cv, dx] = tw[m, :, :, 0, dx].T
                wt[64:128, m, cv, dx] = tw[m, :, :, 1, dx].T
                wt[0:64, m, cv, 3 + dx] = tw[m, :, :, 2, dx].T
    wt = wt.reshape(128, G * 1152).astype(np.float16)
    w5 = np.zeros((128, G * 6), np.float32)
    for m in range(G):
        for dx in range(3):
            w5[0:64, m * 6 + dx] = t5_w[m, 0, :, 0, dx]
            w5[64:128, m * 6 + dx] = t5_w[m, 0, :, 1, dx]
            w5[0:64, m * 6 + 3 + dx] = t5_w[m, 0, :, 2, dx]
    w5 = w5.astype(np.float16)
    bb = np.zeros((64, 39), np.float32)
    aa = np.zeros((64, 39), np.float32)
    bb[:, 0:8] = h1_b.T; aa[:, 0:8] = np.broadcast_to(h1_a, (64, 8))
    bb[:, 8:15] = fus_b.T; aa[:, 8:15] = np.broadcast_to(fus_a, (64, 7))
    bb[:, 15:23] = t2_b.T; aa[:, 15:23] = np.broadcast_to(t2_a, (64, 8))
    bb[:, 23:31] = t3_b.T; aa[:, 23:31] = np.broadcast_to(t3_a, (64, 8))
    bb[:, 31:39] = t4_b.T; aa[:, 31:39] = np.broadcast_to(t4_a, (64, 8))
    bb = np.concatenate([bb, bb], axis=0)   # B-chain ACTs read rows 64:128
    aa = np.concatenate([aa, aa], axis=0)

    in_maps = []
    for c in range(8):
        mmk = np.ones((128, 2), np.float32)
        if c == 0:
            mmk[:, 0] = 0.0
        if c == 7:
            mmk[:, 1] = 0.0
        in_maps.append({"ha": ha[c], "fi": fi[c], "hf": hfd[c], "wf": wf,
                        "wt": wt, "w5": w5, "bb": bb, "aa": aa, "mm": mmk})
    return in_maps


def kernel(x, sample_w, up_w, up_b, h1_w, h1_b, h1_a, fus_w, fus_b, fus_a,
           t2_w, t2_b, t2_a, t3_w, t3_b, t3_a, t4_w, t4_b, t4_a, t5_w, t5_b):
    in_maps = build_in_maps(
        x, sample_w, up_w, up_b, h1_w, h1_b, h1_a, fus_w, fus_b, fus_a,
        t2_w, t2_b, t2_a, t3_w, t3_b, t3_a, t4_w, t4_b, t4_a, t5_w, t5_b)
    results = _run(in_maps)
    out = np.empty((G, 1, 1, H, W), np.float32)
    for c in range(8):
        o = results[c]["o"].reshape(G, 32, 256)
        out[:, 0, 0, 32 * c:32 * c + 32, :] = o
    out += np.asarray(t5_b).reshape(G, 1, 1, 1, 1)
    return out
